# revision 12
# baseline (speedup 1.0000x reference)
"""DeepSeek-MoE transformer block on 8 Trainium2 NeuronCores.

Strategy:
  NEFF1 (attention): head-parallel MLA attention (2 heads/core) in a
  feature-major ("transposed", [feature, token]) layout, AllToAll to
  token-shard the per-head outputs, then output projection + residual.
  Host: router softmax + top-2 + aux loss (from f32 x_mid), per-expert
  capacity-padded gather.
  NEFF2 (MoE): expert-parallel SwiGLU (1 expert/core) over gathered tokens.
  Host: weighted scatter-combine.

Compute dtype: bf16 matmul inputs, f32 accumulation and normalizations.
"""

import os
import numpy as np
import ml_dtypes

import concourse.bass as bass
import concourse.mybir as mybir
import concourse.tile as tile
from concourse.bass_utils import run_bass_kernel_spmd
from concourse.vector_clock import ScopedClock

F32 = mybir.dt.float32
BF16 = mybir.dt.bfloat16
AF = mybir.ActivationFunctionType
NPBF16 = ml_dtypes.bfloat16

NCORES = 8
B, S, D, H = 2, 2048, 1024, 16
DN, DR, DV, KVR = 128, 64, 128, 512
E, TOPK, F = 8, 2, 4096
EPS = 1e-6
THETA = 10000.0
T = B * S               # 4096 tokens
W = T // NCORES         # 512-token window per core
HPC = H // NCORES       # 2 heads per core
SCALE = float(1.0 / np.sqrt(np.float32(DN + DR)))
CAP = 1152              # per-expert token capacity per MoE launch

TRACE = bool(os.environ.get("BASS_KERNEL_TRACE"))
last_profile = {}

# --------------------------------------------------------------------------
# Tile exit-path workaround: this walrus build rejects instructions carrying
# more than one sync wait; Tile's default exit puts every outstanding sem
# wait on the final Drain.  Split them across single-wait NOP spacers.
# --------------------------------------------------------------------------
_PATCHED = False


def _patch_tile():
    global _PATCHED
    if _PATCHED:
        return
    _PATCHED = True

    def _drain_and_barrier(self, tick_clock, wait_clock):
        nc = self.nc
        drain_inst = nc.sync.drain()
        wait_clock.add_sem_waits(
            drain_inst.ins, ScopedClock({None: tick_clock.global_clock})
        )
        si = drain_inst.ins.sync_info
        waits = list(si.on_wait) if si and si.on_wait else []
        if len(waits) > 1:
            si.on_wait = waits[:1]
            for w in waits[1:]:
                nop = nc.sync.nop()
                nsi = nop.ins.sync_info
                if nsi is None:
                    nop.ins.sync_info = mybir.SyncInfo(on_wait=[w], on_update=[])
                else:
                    nsi.on_wait = [w]
        nc.all_engine_barrier()
        assert self.sems is not None
        popped = nc._tile_sem_poison_stack.pop()
        assert popped is self._sem_poison
        nc.clear_and_free_semaphores(list(self.sems.allocated().values()))
        nc.all_engine_barrier()

    tile.TileContext._drain_and_barrier = _drain_and_barrier


def _split_waits(nc):
    """This walrus build allows at most ONE sync wait per instruction.
    Move excess waits onto EventSemaphore spacers inserted just before the
    instruction on the same engine."""
    n = 0
    for f in nc.m.functions:
        for bb in f.blocks:
            insts = list(bb.instructions)
            out = []
            changed = False
            for ins in insts:
                si = ins.sync_info
                waits = list(si.on_wait) if si and si.on_wait else []
                if len(waits) > 1:
                    changed = True
                    for w in waits[:-1]:
                        sp = mybir.InstEventSemaphore(
                            name=f"wsplit-{n}", engine=ins.engine,
                            debug=ins.debug, ins=[], outs=[],
                            sync_info=mybir.SyncInfo(on_wait=[w],
                                                     on_update=[]))
                        n += 1
                        out.append(sp)
                    si.on_wait = [waits[-1]]
                out.append(ins)
            if changed:
                bb.instructions = out
    return n


# --------------------------------------------------------------------------
# NEFF 1: attention (full fp32 — routing requires near-exact x_mid)
# --------------------------------------------------------------------------
def _build_attn():
    _patch_tile()
    nc = bass.Bass("TRN2", target_bir_lowering=False, debug=False,
                   num_devices=NCORES)
    xT = nc.dram_tensor("xT", (D, T), F32, kind="ExternalInput").ap()
    xres = nc.dram_tensor("xres", (D, W), F32, kind="ExternalInput").ap()
    wqn = nc.dram_tensor("wqn", (D, HPC * DN), F32, kind="ExternalInput").ap()
    wqp = nc.dram_tensor("wqp", (D, HPC * DR), F32, kind="ExternalInput").ap()
    wkva = nc.dram_tensor("wkva", (D, KVR + DR), F32, kind="ExternalInput").ap()
    wkvbk = nc.dram_tensor("wkvbk", (KVR, HPC * DN), F32, kind="ExternalInput").ap()
    wkvbv = nc.dram_tensor("wkvbv", (KVR, HPC * DV), F32, kind="ExternalInput").ap()
    wo = nc.dram_tensor("wo", (H * DV, D), F32, kind="ExternalInput").ap()
    cosw = nc.dram_tensor("cosw", (DR, T), F32, kind="ExternalInput").ap()
    sinw = nc.dram_tensor("sinw", (DR, T), F32, kind="ExternalInput").ap()
    maskq = nc.dram_tensor("maskq", (128, 4 * 512), F32, kind="ExternalInput").ap()
    xmid = nc.dram_tensor("xmid", (D, W), F32, kind="ExternalOutput").ap()

    NSL = T // 512      # 8 token slices
    NDC = D // 128      # 8 feature chunks
    NKC = KVR // 128    # 4 kv-lora chunks
    NTT = S // 128      # 16 key tiles per batch

    with tile.TileContext(nc) as tc:
        with tc.tile_pool(name="persist", bufs=1) as pp, \
             tc.tile_pool(name="dramP", bufs=1, space="DRAM") as dramP:
            # SBUF-persistent (small)
            QTp0 = pp.tile([DR, T], F32, tag="QTp0")
            QTp1 = pp.tile([DR, T], F32, tag="QTp1")
            KpT = pp.tile([DR, T], F32, tag="KpT")
            ones_bf = pp.tile([128, 1], BF16, tag="ones_bf")
            ones_f = pp.tile([128, 1], F32, tag="ones_f")
            ones1p = pp.tile([1, 128], F32, tag="ones1p")
            eps_sb = pp.tile([1, 1], F32, tag="eps")
            nc.vector.memset(ones_bf[:], 1.0)
            nc.vector.memset(ones_f[:], 1.0)
            nc.vector.memset(ones1p[:], 1.0)
            nc.vector.memset(eps_sb[:], EPS)
            QTp = (QTp0, QTp1)
            # DRAM-resident big activations
            QTn_d = dramP.tile([HPC, 128, T], F32, tag="QTn_d")
            KnT_d = dramP.tile([HPC, 128, T], F32, tag="KnT_d")
            V_d = dramP.tile([T // 128, 128, HPC * DV], F32, tag="V_d")
            OT_d = dramP.tile([HPC, 128, T], F32, tag="OT_d")
            ckv_send = dramP.tile([KVR + DR, W], F32, tag="ckv_send")
            agckv = dramP.tile([NCORES, KVR + DR, W], F32, tag="agckv")

            # ------------------- phase A: projections -------------------
            with tc.tile_pool(name="wA", bufs=1) as wA, \
                 tc.tile_pool(name="pa", bufs=2) as pa, \
                 tc.tile_pool(name="pa1", bufs=1) as pa1, \
                 tc.tile_pool(name="psA", bufs=2, space="PSUM") as psA, \
                 tc.tile_pool(name="psS", bufs=2, space="PSUM") as psS:

                wqn_sb = wA.tile([128, NDC, HPC * DN], F32, tag="wqn")
                wqp_sb = wA.tile([128, NDC, HPC * DR], F32, tag="wqp")
                wkvbk_sb = wA.tile([128, NKC, HPC * DN], F32, tag="wkvbk")
                wkvbv_sb = wA.tile([128, NKC, HPC * DV], F32, tag="wkvbv")
                for dc in range(NDC):
                    r = slice(dc * 128, dc * 128 + 128)
                    nc.sync.dma_start(wqn_sb[:, dc, :], wqn[r, :])
                    nc.sync.dma_start(wqp_sb[:, dc, :], wqp[r, :])
                for kc in range(NKC):
                    r = slice(kc * 128, kc * 128 + 128)
                    nc.sync.dma_start(wkvbk_sb[:, kc, :], wkvbk[r, :])
                    nc.sync.dma_start(wkvbv_sb[:, kc, :], wkvbv[r, :])

                def rmsnorm_bcast(src_tiles, nchunk, dim, tag):
                    """src_tiles(i) -> [128, 512] f32 AP; returns [128,512]
                    f32 SBUF tile with 1/rms broadcast to all partitions."""
                    ss = psA.tile([1, 512], F32, tag="psa")
                    for i in range(nchunk):
                        sq = pa.tile([128, 512], F32, tag="sq")
                        nc.vector.tensor_mul(sq[:], src_tiles(i), src_tiles(i))
                        nc.tensor.matmul(ss[:], ones_f[:], sq[:],
                                         start=(i == 0), stop=(i == nchunk - 1))
                    sqv = pa.tile([1, 512], F32, tag="sqv")
                    nc.scalar.activation(sqv[:], ss[:], AF.Sqrt,
                                         bias=eps_sb[:], scale=1.0 / dim)
                    rstd = pa.tile([1, 512], F32, tag="rstd")
                    nc.vector.reciprocal(rstd[:], sqv[:])
                    bc = psA.tile([128, 512], F32, tag="psa")
                    nc.tensor.matmul(bc[:], ones1p[:], rstd[:],
                                     start=True, stop=True)
                    bcs = pa.tile([128, 512], F32, tag="bcs")
                    nc.scalar.copy(bcs[:], bc[:])
                    return bcs

                def rope(dst, src_f32, cols, tag):
                    """src_f32: [64, 512] f32 SBUF; dst: [64, 512] AP."""
                    cos_t = pa.tile([DR, 512], F32, tag="cost")
                    nc.sync.dma_start(cos_t[:], cosw[:, cols])
                    sin_t = pa.tile([DR, 512], F32, tag="sint")
                    nc.sync.dma_start(sin_t[:], sinw[:, cols])
                    rot = pa.tile([DR, 512], F32, tag="rot")
                    nc.sync.dma_start(rot[0:32, :], src_f32[32:64, :])
                    nc.sync.dma_start(rot[32:64, :], src_f32[0:32, :])
                    t1 = pa.tile([DR, 512], F32, tag="t1")
                    nc.vector.tensor_mul(t1[:], src_f32[:], cos_t[:])
                    nc.vector.tensor_mul(rot[:], rot[:], sin_t[:])
                    nc.vector.tensor_add(dst, t1[:], rot[:])

                # --- A0: my-window ckv slice + AllGather (dedups the big
                # ckv projection across cores) ---
                wkva_pool = tc.tile_pool(name="wAckv", bufs=1)
                wAc = wkva_pool.__enter__()
                wkva_sb = wAc.tile([128, NDC, KVR + DR], F32, tag="wkva")
                for dc in range(NDC):
                    r = slice(dc * 128, dc * 128 + 128)
                    nc.sync.dma_start(wkva_sb[:, dc, :], wkva[r, :])
                xtm = pa1.tile([128, NDC, 512], F32, tag="xt")
                for dc in range(NDC):
                    nc.sync.dma_start(xtm[:, dc, :],
                                      xres[dc * 128:dc * 128 + 128, :])
                bcsm = rmsnorm_bcast(lambda i: xtm[:, i, :], NDC, D, "h")
                htm = pa1.tile([128, NDC, 512], F32, tag="ht")
                for dc in range(NDC):
                    nc.vector.tensor_mul(htm[:, dc, :], xtm[:, dc, :], bcsm[:])
                for kc in range(NKC + 1):
                    kk = slice(kc * 128, min(KVR + DR, kc * 128 + 128))
                    npart = kk.stop - kk.start
                    cps = psS.tile([128, 512], F32, tag="pss")
                    for dc in range(NDC):
                        nc.tensor.matmul(cps[:npart, :], wkva_sb[:, dc, kk],
                                         htm[:, dc, :],
                                         start=(dc == 0), stop=(dc == NDC - 1))
                    cse = pa.tile([128, 512], F32, tag="evac")
                    nc.scalar.copy(cse[:npart, :], cps[:npart, :])
                    nc.sync.dma_start(ckv_send[kk, :], cse[:npart, :])
                nc.gpsimd.collective_compute(
                    "AllGather", mybir.AluOpType.bypass,
                    replica_groups=[list(range(NCORES))],
                    ins=[ckv_send.opt()], outs=[agckv.opt()])
                wkva_pool.__exit__(None, None, None)

                # --- A1: h + q projections for all slices ---
                for sl in range(NSL):
                    cols = slice(sl * 512, sl * 512 + 512)
                    xt = pa1.tile([128, NDC, 512], F32, tag="xt")
                    for dc in range(NDC):
                        nc.sync.dma_start(xt[:, dc, :],
                                          xT[dc * 128:dc * 128 + 128, cols])
                    bcs = rmsnorm_bcast(lambda i: xt[:, i, :], NDC, D, "h")
                    ht = pa1.tile([128, NDC, 512], F32, tag="ht")
                    for dc in range(NDC):
                        nc.vector.tensor_mul(ht[:, dc, :], xt[:, dc, :], bcs[:])
                    for h in range(HPC):
                        qn = psS.tile([128, 512], F32, tag="pss")
                        for dc in range(NDC):
                            nc.tensor.matmul(
                                qn[:], wqn_sb[:, dc, h * DN:(h + 1) * DN],
                                ht[:, dc, :],
                                start=(dc == 0), stop=(dc == NDC - 1))
                        qns = pa.tile([128, 512], F32, tag="evac")
                        nc.scalar.copy(qns[:], qn[:])
                        nc.sync.dma_start(QTn_d[h, :, cols], qns[:])
                        qp = psS.tile([DR, 512], F32, tag="pss")
                        for dc in range(NDC):
                            nc.tensor.matmul(
                                qp[:], wqp_sb[:, dc, h * DR:(h + 1) * DR],
                                ht[:, dc, :],
                                start=(dc == 0), stop=(dc == NDC - 1))
                        qpf = pa.tile([DR, 512], F32, tag="pef")
                        nc.scalar.copy(qpf[:], qp[:])
                        rope(QTp[h][:, cols], qpf, cols, "q")

                # --- A2: per-slice c_kv norm + K/V projections ---
                for sl in range(NSL):
                    cols = slice(sl * 512, sl * 512 + 512)
                    ckvf = pa1.tile([128, NKC, 512], F32, tag="ckvf")
                    for kc in range(NKC):
                        nc.sync.dma_start(ckvf[:, kc, :],
                                          agckv[sl, kc * 128:kc * 128 + 128, :])
                    kpef = pa.tile([DR, 512], F32, tag="pef")
                    nc.sync.dma_start(kpef[:], agckv[sl, KVR:KVR + DR, :])
                    rope(KpT[:, cols], kpef, cols, "k")
                    bcs2 = rmsnorm_bcast(lambda i: ckvf[:, i, :], NKC, KVR, "c")
                    cn = pa1.tile([128, NKC, 512], F32, tag="cn")
                    for kc in range(NKC):
                        nc.vector.tensor_mul(cn[:, kc, :], ckvf[:, kc, :],
                                             bcs2[:])
                    for h in range(HPC):
                        kn = psS.tile([128, 512], F32, tag="pss")
                        for kc in range(NKC):
                            nc.tensor.matmul(
                                kn[:], wkvbk_sb[:, kc, h * DN:(h + 1) * DN],
                                cn[:, kc, :],
                                start=(kc == 0), stop=(kc == NKC - 1))
                        kns = pa.tile([128, 512], F32, tag="evac")
                        nc.scalar.copy(kns[:], kn[:])
                        nc.sync.dma_start(KnT_d[h, :, cols], kns[:])
                    for tt in range(4):
                        vps = psS.tile([128, HPC * DV], F32, tag="pss")
                        for kc in range(NKC):
                            nc.tensor.matmul(
                                vps[:], cn[:, kc, tt * 128:tt * 128 + 128],
                                wkvbv_sb[:, kc, :],
                                start=(kc == 0), stop=(kc == NKC - 1))
                        vs = pa.tile([128, HPC * DV], F32, tag="evac")
                        nc.scalar.copy(vs[:], vps[:])
                        nc.sync.dma_start(V_d[sl * 4 + tt, :, :], vs[:])

            # ------------------- phase B: attention -------------------
            with tc.tile_pool(name="pbm", bufs=1) as pbm, \
                 tc.tile_pool(name="pb1", bufs=2) as pb1, \
                 tc.tile_pool(name="pb", bufs=3) as pb, \
                 tc.tile_pool(name="psO", bufs=2, space="PSUM") as psO, \
                 tc.tile_pool(name="psB", bufs=2, space="PSUM") as psB, \
                 tc.tile_pool(name="psS2", bufs=2, space="PSUM") as psS2:
                mask_sb = pbm.tile([128, 4, 512], F32, tag="mask")
                nc.sync.dma_start(mask_sb[:],
                                  maskq.rearrange("p (j f) -> p j f", j=4))
                for b in range(B):
                    Vb = pb1.tile([128, NTT, HPC * DV], F32, tag="Vb")
                    nc.sync.dma_start(
                        Vb[:], V_d[b * NTT:(b + 1) * NTT]
                        .rearrange("n p f -> p n f"))
                    for h in range(HPC):
                        bcols = slice(b * S, (b + 1) * S)
                        Kh = pb1.tile([128, S], F32, tag="Kh")
                        nc.sync.dma_start(Kh[:], KnT_d[h, :, bcols])
                        Qh = pb1.tile([128, S], F32, tag="Qh")
                        nc.sync.dma_start(Qh[:], QTn_d[h, :, bcols])
                        for qs in range(S // 512):
                            qcol = slice(b * S + qs * 512,
                                         b * S + qs * 512 + 512)
                            lq = slice(qs * 512, qs * 512 + 512)
                            nkt = 4 * (qs + 1)
                            ot_ps = psO.tile([128, 512], F32, tag="ot")
                            acc = pb.tile([128, 512], F32, tag="acc")
                            for kt in range(nkt):
                                lk = slice(kt * 128, kt * 128 + 128)
                                kcols = slice(b * S + kt * 128,
                                              b * S + kt * 128 + 128)
                                s_ps = psS2.tile([128, 512], F32, tag="s")
                                nc.tensor.matmul(s_ps[:], Kh[:, lk],
                                                 Qh[:, lq],
                                                 start=True, stop=False)
                                nc.tensor.matmul(s_ps[:], KpT[:, kcols],
                                                 QTp[h][:, qcol],
                                                 start=False, stop=True)
                                p_sb = pb.tile([128, 512], F32, tag="p")
                                nc.scalar.activation(p_sb[:], s_ps[:], AF.Exp,
                                                     scale=SCALE)
                                if kt >= qs * 4:
                                    nc.vector.tensor_mul(
                                        p_sb[:], p_sb[:],
                                        mask_sb[:, kt - qs * 4, :])
                                if kt == 0:
                                    nc.vector.tensor_copy(acc[:], p_sb[:])
                                else:
                                    nc.vector.tensor_add(acc[:], acc[:],
                                                         p_sb[:])
                                nc.tensor.matmul(
                                    ot_ps[:], Vb[:, kt, h * DV:(h + 1) * DV],
                                    p_sb[:],
                                    start=(kt == 0), stop=(kt == nkt - 1))
                            den = psB.tile([1, 512], F32, tag="psb")
                            nc.tensor.matmul(den[:], ones_f[:], acc[:],
                                             start=True, stop=True)
                            rc = pb.tile([1, 512], F32, tag="rc")
                            nc.vector.reciprocal(rc[:], den[:])
                            bcp = psB.tile([128, 512], F32, tag="psb")
                            nc.tensor.matmul(bcp[:], ones1p[:], rc[:],
                                             start=True, stop=True)
                            bcb = pb.tile([128, 512], F32, tag="bcb")
                            nc.scalar.copy(bcb[:], bcp[:])
                            otn = pb.tile([128, 512], F32, tag="otn")
                            nc.vector.tensor_mul(otn[:], ot_ps[:], bcb[:])
                            nc.sync.dma_start(OT_d[h, :, qcol], otn[:])

            # ------------------- phase C: A2A + out proj -------------------
            with tc.tile_pool(name="dramC", bufs=1, space="DRAM") as dramC, \
                 tc.tile_pool(name="pc", bufs=1) as pc, \
                 tc.tile_pool(name="pc2", bufs=2) as pc2, \
                 tc.tile_pool(name="psC", bufs=2, space="PSUM") as psC:
                send = dramC.tile([NCORES, HPC * 128, 512], F32, tag="send")
                recv = dramC.tile([NCORES, HPC * 128, 512], F32, tag="recv")
                for wd in range(NCORES):
                    for h in range(HPC):
                        otb = pc2.tile([128, 512], F32, tag="otb")
                        nc.sync.dma_start(
                            otb[:], OT_d[h, :, wd * 512:wd * 512 + 512])
                        nc.sync.dma_start(
                            send[wd, h * 128:(h + 1) * 128, :], otb[:])
                nc.gpsimd.collective_compute(
                    "AllToAll", mybir.AluOpType.bypass,
                    replica_groups=[list(range(NCORES))],
                    ins=[send.opt()], outs=[recv.opt()])
                otf = pc.tile([128, H, 512], F32, tag="otf")
                for i in range(NCORES):
                    for hh in range(HPC):
                        nc.sync.dma_start(otf[:, HPC * i + hh, :],
                                          recv[i, hh * 128:(hh + 1) * 128, :])
                wo_sb = pc.tile([128, H * DV // 128, D], F32, tag="wo")
                for ch in range(H * DV // 128):
                    nc.sync.dma_start(wo_sb[:, ch, :],
                                      wo[ch * 128:ch * 128 + 128, :])
                for dt in range(NDC):
                    ops = psC.tile([128, 512], F32, tag="o")
                    for ch in range(H * DV // 128):
                        nc.tensor.matmul(
                            ops[:], wo_sb[:, ch, dt * 128:dt * 128 + 128],
                            otf[:, ch, :],
                            start=(ch == 0), stop=(ch == H * DV // 128 - 1))
                    xr = pc2.tile([128, 512], F32, tag="xr")
                    nc.sync.dma_start(xr[:], xres[dt * 128:dt * 128 + 128, :])
                    xm = pc2.tile([128, 512], F32, tag="xm")
                    nc.vector.tensor_add(xm[:], ops[:], xr[:])
                    nc.sync.dma_start(xmid[dt * 128:dt * 128 + 128, :], xm[:])
    _split_waits(nc)
    return nc


# --------------------------------------------------------------------------
# NEFF 2: MoE expert (SwiGLU) over CAP gathered tokens
# --------------------------------------------------------------------------
def _build_moe():
    _patch_tile()
    nc = bass.Bass("TRN2", target_bir_lowering=False, debug=False,
                   num_devices=NCORES)
    h2t = nc.dram_tensor("h2t", (D, CAP), BF16, kind="ExternalInput").ap()
    wg = nc.dram_tensor("wg", (D // 128, 128, F), BF16, kind="ExternalInput").ap()
    wu = nc.dram_tensor("wu", (D // 128, 128, F), BF16, kind="ExternalInput").ap()
    wd = nc.dram_tensor("wd", (D // 128, 128, F // 128, 128), BF16,
                        kind="ExternalInput").ap()
    y = nc.dram_tensor("y", (D, CAP), F32, kind="ExternalOutput").ap()

    NDC = D // 128      # 8
    NFC = F // 128      # 32
    CS = []             # ragged capacity slices (<=512 wide)
    c0 = 0
    while c0 < CAP:
        cw = min(512, CAP - c0)
        CS.append((c0, cw))
        c0 += cw

    with tile.TileContext(nc) as tc:
        with tc.tile_pool(name="ph", bufs=1) as ph, \
             tc.tile_pool(name="pA", bufs=1) as pA:
            h2_sb = ph.tile([128, NDC, CAP], BF16, tag="h2")
            for dc in range(NDC):
                nc.sync.dma_start(h2_sb[:, dc, :],
                                  h2t[dc * 128:dc * 128 + 128, :])
            A_sb = pA.tile([128, NFC, CAP], BF16, tag="A")

            with tc.tile_pool(name="pw", bufs=2) as pw, \
                 tc.tile_pool(name="pg", bufs=3) as pg, \
                 tc.tile_pool(name="psG", bufs=4, space="PSUM") as psG:
                for fg in range(8):
                    fcols = slice(fg * 512, fg * 512 + 512)
                    wg_sb = pw.tile([128, NDC, 512], BF16, tag="wg")
                    wu_sb = pw.tile([128, NDC, 512], BF16, tag="wu")
                    for dc in range(NDC):
                        nc.sync.dma_start(wg_sb[:, dc, :], wg[dc, :, fcols])
                        nc.sync.dma_start(wu_sb[:, dc, :], wu[dc, :, fcols])
                    for f4 in range(4):
                        ft = fg * 4 + f4
                        fs = slice(f4 * 128, f4 * 128 + 128)
                        for (c0, cw) in CS:
                            ccols = slice(c0, c0 + cw)
                            gps = psG.tile([128, 512], F32, tag="psg")
                            for dc in range(NDC):
                                nc.tensor.matmul(
                                    gps[:, :cw], wg_sb[:, dc, fs],
                                    h2_sb[:, dc, ccols],
                                    start=(dc == 0), stop=(dc == NDC - 1))
                            gt = pg.tile([128, 512], BF16, tag="gt")
                            nc.scalar.activation(gt[:, :cw], gps[:, :cw],
                                                 AF.Silu)
                            ups = psG.tile([128, 512], F32, tag="psg")
                            for dc in range(NDC):
                                nc.tensor.matmul(
                                    ups[:, :cw], wu_sb[:, dc, fs],
                                    h2_sb[:, dc, ccols],
                                    start=(dc == 0), stop=(dc == NDC - 1))
                            nc.vector.tensor_mul(A_sb[:, ft, ccols],
                                                 gt[:, :cw], ups[:, :cw])

            with tc.tile_pool(name="pw2", bufs=2) as pw2, \
                 tc.tile_pool(name="pe2", bufs=3) as pe2, \
                 tc.tile_pool(name="psD", bufs=3, space="PSUM") as psD:
                for dt in range(NDC):
                    wd_sb = pw2.tile([128, NFC, 128], BF16, tag="wd")
                    nc.sync.dma_start(wd_sb[:], wd[dt])
                    for (c0, cw) in CS:
                        ccols = slice(c0, c0 + cw)
                        dps = psD.tile([128, 512], F32, tag="d")
                        for fc in range(NFC):
                            nc.tensor.matmul(dps[:, :cw], wd_sb[:, fc, :],
                                             A_sb[:, fc, ccols],
                                             start=(fc == 0),
                                             stop=(fc == NFC - 1))
                        ysb = pe2.tile([128, 512], F32, tag="y")
                        nc.scalar.copy(ysb[:, :cw], dps[:, :cw])
                        nc.sync.dma_start(y[dt * 128:dt * 128 + 128, ccols],
                                          ysb[:, :cw])
    _split_waits(nc)
    return nc


_attn_nc = None
_moe_nc = None


def _get_attn_nc():
    global _attn_nc
    if _attn_nc is None:
        _attn_nc = _build_attn()
    return _attn_nc


def _get_moe_nc():
    global _moe_nc
    if _moe_nc is None:
        _moe_nc = _build_moe()
    return _moe_nc


def _bf(a):
    return np.ascontiguousarray(np.asarray(a, np.float32)).astype(NPBF16)


def _rope_tables():
    inv = (1.0 / (np.float32(THETA) **
                  (np.arange(0, DR, 2, dtype=np.float32) / np.float32(DR))))
    ang = np.arange(S, dtype=np.float32)[:, None] * inv[None, :]
    emb = np.concatenate([ang, ang], axis=-1)          # [S, DR]
    return np.cos(emb, dtype=np.float32), np.sin(emb, dtype=np.float32)


def kernel(x, ln1_w, ln2_w, wq, wkv_a, kv_norm_w, wkv_b, wo,
           w_router, w_gate, w_up, w_down):
    x = np.asarray(x, np.float32)
    ln1_w = np.asarray(ln1_w, np.float32)
    ln2_w = np.asarray(ln2_w, np.float32)

    # ---- host prep: fold elementwise norm weights into following matmuls
    xT = np.ascontiguousarray(x.reshape(T, D).T)                    # [D, T]
    wq_e = (ln1_w[:, None] * np.asarray(wq, np.float32)).reshape(D, H, DN + DR)
    wkva_e = ln1_w[:, None] * np.asarray(wkv_a, np.float32)
    wkvb_e = (np.asarray(kv_norm_w, np.float32)[:, None]
              * np.asarray(wkv_b, np.float32)).reshape(KVR, H, DN + DV)

    cos, sin = _rope_tables()
    f32c = lambda a: np.ascontiguousarray(a, dtype=np.float32)
    cosw = f32c(np.concatenate([cos.T, cos.T], axis=1))             # [DR, T]
    sw = sin.T.copy()
    sw[:DR // 2] *= -1.0
    sinw = f32c(np.concatenate([sw, sw], axis=1))

    p_idx = np.arange(128)[:, None]
    f_idx = np.arange(512)[None, :]
    maskq = f32c(np.concatenate(
        [(j * 128 + p_idx <= f_idx).astype(np.float32) for j in range(4)],
        axis=1))                                                     # [128, 2048]

    wkva_b = f32c(wkva_e)
    wo_b = f32c(wo)

    in_maps = []
    for c in range(NCORES):
        hs = slice(HPC * c, HPC * c + HPC)
        in_maps.append({
            "xT": xT,
            "xres": f32c(xT[:, c * W:(c + 1) * W]),
            "wqn": f32c(wq_e[:, hs, :DN].reshape(D, HPC * DN)),
            "wqp": f32c(wq_e[:, hs, DN:].reshape(D, HPC * DR)),
            "wkva": wkva_b,
            "wkvbk": f32c(wkvb_e[:, hs, :DN].reshape(KVR, HPC * DN)),
            "wkvbv": f32c(wkvb_e[:, hs, DN:].reshape(KVR, HPC * DV)),
            "wo": wo_b,
            "cosw": cosw,
            "sinw": sinw,
            "maskq": maskq,
        })

    res1 = run_bass_kernel_spmd(_get_attn_nc(), in_maps,
                                core_ids=list(range(NCORES)), trace=TRACE)
    last_profile["attn_ns"] = res1.exec_time_ns
    xmidT = np.concatenate([res1.results[c]["xmid"] for c in range(NCORES)],
                           axis=1)                                   # [D, T]
    xmid = np.ascontiguousarray(xmidT.T, dtype=np.float64)           # [T, D]

    # ---- host routing (f64) ----
    var = np.mean(xmid * xmid, axis=-1, keepdims=True)
    h2 = xmid / np.sqrt(var + EPS) * ln2_w.astype(np.float64)
    logits = h2 @ np.asarray(w_router, np.float64)
    logits -= logits.max(axis=-1, keepdims=True)
    probs = np.exp(logits)
    probs /= probs.sum(axis=-1, keepdims=True)
    topi = np.argsort(-probs, axis=-1, kind="stable")[:, :TOPK]      # [T, 2]
    topv = np.take_along_axis(probs, topi, axis=-1)
    wts = topv / topv.sum(axis=-1, keepdims=True)                    # [T, 2]

    counts = np.bincount(topi.reshape(-1), minlength=E)
    f_frac = counts.astype(np.float64) / float(T * TOPK)
    P_mean = probs.mean(axis=0)
    aux_loss = np.float32(E * np.sum(f_frac.astype(np.float32)
                                     * P_mean.astype(np.float32)))

    # ---- MoE: expert-parallel, capacity-padded ----
    h2f = h2.astype(np.float32)
    wg_r = [None] * E
    wu_r = [None] * E
    wd_r = [None] * E
    for e in range(E):
        wg_r[e] = _bf(np.asarray(w_gate[e], np.float32)).reshape(D // 128, 128, F)
        wu_r[e] = _bf(np.asarray(w_up[e], np.float32)).reshape(D // 128, 128, F)
        wd_r[e] = np.ascontiguousarray(
            _bf(np.asarray(w_down[e], np.float32))
            .reshape(F // 128, 128, D // 128, 128)
            .transpose(2, 1, 0, 3))                    # [8, 128, 32, 128]

    tok_idx = [np.where((topi == e).any(axis=1))[0] for e in range(E)]
    slot = [np.where(topi[tok_idx[e]][:, 0] == e, 0, 1) for e in range(E)]
    wts_e = [wts[tok_idx[e], slot[e]] for e in range(E)]

    ff = np.zeros((T, D), np.float32)
    max_rounds = (max(len(t) for t in tok_idx) + CAP - 1) // CAP
    for rnd in range(max_rounds):
        in2 = []
        for e in range(E):
            idx = tok_idx[e][rnd * CAP:(rnd + 1) * CAP]
            h2e = np.zeros((CAP, D), np.float32)
            h2e[:len(idx)] = h2f[idx]
            in2.append({
                "h2t": np.ascontiguousarray(h2e.T).astype(NPBF16),
                "wg": wg_r[e], "wu": wu_r[e], "wd": wd_r[e],
            })
        res2 = run_bass_kernel_spmd(_get_moe_nc(), in2,
                                    core_ids=list(range(NCORES)), trace=TRACE)
        last_profile[f"moe_ns_{rnd}"] = res2.exec_time_ns

        for e in range(E):
            idx = tok_idx[e][rnd * CAP:(rnd + 1) * CAP]
            w_e = wts_e[e][rnd * CAP:(rnd + 1) * CAP]
            ye = res2.results[e]["y"][:, :len(idx)]                  # [D, n]
            ff[idx] += (w_e[:, None] * ye.T).astype(np.float32)

    last_profile["xmid"] = xmid.astype(np.float32)
    last_profile["ff"] = ff
    last_profile["topi"] = topi
    last_profile["wts"] = wts
    last_profile["counts"] = counts
    out = (xmid.astype(np.float32) + ff).reshape(B, S, D)
    return out, aux_loss
